# revision 7
# baseline (speedup 1.0000x reference)
"""Deformable Conv1D kernel v2 for Trainium2 (8 NeuronCores, Bass/Tile).

j-partition layout. Per core (512 output rows, j-window of 640 = 5 blocks
of 128 on partitions):

  off[t,q]  = relu(conv(x)[jl]) - x[jl],  jl = 128q + t - 64   (PE matmul,
              block-diagonal weights, fp32r -> [128,5] PSUM, one DVE stt)
  w[t,q,c]  = (c - t - 6) + off[t,q]      (C3T iota constant + per-partition
              scalar adds; the 128q term cancels exactly)
  A[t,q,c]  = g(clamp(w,0,4)) = c0*rc + c1*relu(rc-1) + c2*relu(rc-2)
              + c3*relu(rc-3)             (flipped hat basis, c_k from W on
              host; g==0 outside the band by exact cancellation)
  y rows    = per-q PE matmuls  out[1,W] += xcol_q^T-weighted A columns,
              accumulated into 4 pre-zeroed [1,144] PSUM row tiles at the
              right free offsets; band i-j in [-8,8] covered exactly.

The band evaluation runs as three single-uop custom DVE ops (DEFORM_U1 /
DEFORM_TAP), each a full-rate fused pass; only the consumed flat columns
[64, 656) of the (q, c) space are computed.  Inputs arrive as two tiny
parallel DMAs (conv operands [20,134] on Sync, xcol+ck [128,9] on the
Activation queue); C3T is a gpsimd iota.  Output is a [1,512] SBUF row
DMA'd by the Activation engine in program order behind its last copy.
"""

import sys

for _p in ("/opt/trn_rl_repo",):
    if _p not in sys.path:
        sys.path.insert(0, _p)

import numpy as np

import concourse.bass as bass
import concourse.tile as tile
from concourse import bacc, mybir
from concourse import dve_ops as _dve_ops
from concourse.bass_utils import run_bass_kernel_spmd
from concourse.dve_ops import DveOp
from concourse.dve_spec import C0, C1, C2, One, Spec, Src0, Src1, minn, relu

# Fused custom-DVE ops (each lowers to a single uop -> one full-rate pass):
#   DEFORM_U1:  out = s0*rc + s1*relu(rc - 1),    rc = min(in0, imm2)
#   DEFORM_TAP: out = in1 + s0*relu(min(in0, imm2) - s1)
_rc4 = minn(Src0, C2)
DEFORM_U1 = DveOp(
    "DEFORM_U1",
    Spec(
        body=C0 * _rc4 + C1 * relu(_rc4 - One),
        reference=lambda in0, in1, s0, s1, imm2: (
            lambda rc: (s0 * rc + s1 * np.maximum(rc - 1, 0)).astype(np.float32)
        )(np.minimum(in0, imm2)),
    ),
    subdim=False,
    uops_sha={"v3": "d576886c8dcf2626", "v4": "14bd2f5069c80a43"},
)
DEFORM_TAP = DveOp(
    "DEFORM_TAP",
    Spec(
        body=Src1 + C0 * relu(minn(Src0, C2) - C1),
        reference=lambda in0, in1, s0, s1, imm2: (
            in1 + s0 * np.maximum(np.minimum(in0, imm2) - s1, 0)
        ).astype(np.float32),
    ),
    subdim=False,
    uops_sha={"v3": "633be38f6408f71e", "v4": "be509e707f813d31"},
)


def _register(op):
    if op.name not in _dve_ops._SUB_OPCODE_FOR_NAME:
        _dve_ops.OPS.append(op)
        _dve_ops.CUSTOM_DVE_SPECS[op.name] = op.spec
        _dve_ops._SUB_OPCODE_FOR_NAME[op.name] = (
            max(_dve_ops._SUB_OPCODE_FOR_NAME.values()) + 1)
        assert _dve_ops._SUB_OPCODE_FOR_NAME[op.name] < 0x20


_register(DEFORM_U1)
_register(DEFORM_TAP)

F32 = mybir.dt.float32
F32R = mybir.dt.float32r
ALU = mybir.AluOpType
ACTF = mybir.ActivationFunctionType

N = 4096
NCORES = 8
ROWS = N // NCORES   # 512
P = 128
NQ = 5               # j blocks per core (window 640)
WB = 144             # per-block i-window width
F = 287              # packed input columns

# column layout of the packed input.  [0:138] is DMA'd into an f32r tile
# (PE matmul operands must be produced as f32r per the BIR verifier);
# [138:286] into a plain f32 tile.
C_XS = 0        # [0:128]   conv lhsT rows (partitions 0..19)
C_CWD = 128     # [128:134] block-diag conv weights, padded to 6 cols (fp32r
                #           moving operand needs an even innermost count)
C_XCOL = 134    # [134:139] x column per j-block
NR = 139        # f32r section width
C_CK = 139      # [139:143] flipped-basis coefficients c0..c3 (replicated)
C_C3T = 143     # [143:287] C3T[t,c] = c - t - 6


def _emit(tc, nc, pk1_d, pk2_d, y_d):
    with (
        tc.tile_pool(name="const", bufs=1) as const,
        tc.tile_pool(name="work", bufs=1) as work,
        tc.tile_pool(name="psum", bufs=1, space="PSUM") as psum,
    ):
        # two tiny input DMAs on separate queues: conv operands (10.7KB,
        # Sync) and xcol+ck (4.6KB, DVE-issued, first in its stream).
        # C3T is generated on-device (gpsimd iota, off the critical path).
        PKR2 = const.tile([P, 9], F32R)
        nc.scalar.dma_start(PKR2[:], pk2_d[:, :].bitcast(F32R))
        PKR1 = const.tile([20, 134], F32R)
        nc.sync.dma_start(PKR1[:], pk1_d[:, :].bitcast(F32R))
        XS = PKR1[:, 0:128]
        cwd = PKR1[:, 128:134]
        xcol = PKR2[:, 0:NQ]
        xcolf = xcol.bitcast(F32)
        ck = [PKR2[:, NQ + k:NQ + k + 1].bitcast(F32) for k in range(4)]
        C3Tt = const.tile([P, WB], F32)
        nc.gpsimd.iota(C3Tt[:], pattern=[[1, WB]], base=-6,
                       channel_multiplier=-1,
                       allow_small_or_imprecise_dtypes=True)
        C3T = C3Tt[:]

        bm2 = const.tile([P, 1], F32)
        nc.vector.memset(bm2[:], -2.0)
        bm3 = const.tile([P, 1], F32)
        nc.vector.memset(bm3[:], -3.0)
        # dummy activation with no data deps: hoists the ACT table load to
        # the head of the Scalar stream (runs during the input-DMA wait)
        atwarm = const.tile([P, 1], F32)
        nc.scalar.activation(atwarm[:], bm2[:], ACTF.Relu, bias=bm3[:])

        psS = psum.tile([P, 6], F32, tag="psS")
        rowt = [psum.tile([1, WB], F32, tag=f"row{m}", name=f"row{m}")
                for m in range(4)]
        for m in range(4):
            nc.vector.memset(rowt[m][:], 0.0)

        # conv1d offsets: psS[t, q] = sum_c cw[c] * xs_c(jl) + cb  (fp32r)
        nc.tensor.matmul(psS[:], XS, cwd, start=True, stop=True)
        offc = work.tile([P, NQ], F32, tag="offc")
        nc.vector.scalar_tensor_tensor(offc[:], psS[:, 0:NQ], 0.0, xcolf,
                                       ALU.max, ALU.subtract)

        # r0 = relu(C3T + off_q), clamped to 4 in rc.  Only flat columns
        # [64, 656) of the [720] (q, c) space are ever consumed (q0 needs
        # c in [64:144), q4 needs [0:80)), so the edge blocks and the wide
        # fused ops are trimmed to that contiguous range.
        r0 = work.tile([P, NQ, WB], F32, tag="r0")
        nc.scalar.activation(r0[:, 0, 64:144], C3T[:, 64:144], ACTF.Relu,
                             bias=offc[:, 0:1])
        nc.scalar.activation(r0[:, 1, :], C3T, ACTF.Relu, bias=offc[:, 1:2])
        for q in (2, 3):
            nc.vector.tensor_scalar(r0[:, q, :], C3T, offc[:, q:q + 1], 0.0,
                                    ALU.add, ALU.max)
        nc.vector.tensor_scalar(r0[:, 4, 0:80], C3T[:, 0:80], offc[:, 4:5],
                                0.0, ALU.add, ALU.max)

        def _flat(t):
            a = t[:]
            return bass.AP(a.tensor, a.offset + 64,
                           [[a.ap[0][0], P], [1, NQ * WB - 128]])

        u1 = work.tile([P, NQ, WB], F32, tag="u1")
        nc.vector._custom_dve(DEFORM_U1, out=_flat(u1), in0=_flat(r0),
                              s0=ck[0], s1=ck[1], imm2=4.0)
        u2 = work.tile([P, NQ, WB], F32, tag="u2")
        nc.vector._custom_dve(DEFORM_TAP, out=_flat(u2), in0=_flat(r0),
                              in1=_flat(u1), s0=ck[2], s1=2.0, imm2=4.0)
        # final tap split per q so the PE matmuls pipeline with the tail;
        # q=0 / q=4 only need the A columns their matmuls read
        A = work.tile([P, NQ, WB], F32R, tag="A")
        tapcols = {0: (64, 144), 4: (0, 80)}
        for q in range(NQ):
            c0_, c1_ = tapcols.get(q, (0, WB))
            nc.vector._custom_dve(DEFORM_TAP, out=A[:, q, c0_:c1_],
                                  in0=r0[:, q, c0_:c1_],
                                  in1=u2[:, q, c0_:c1_],
                                  s0=ck[3], s1=3.0, imm2=4.0)

        # y row-tile accumulation.  Window q col c -> i_loc = 128q - 72 + c;
        # tile m covers i_loc in [128m - 8, 128m + 136).
        #   q -> tile m=q:   A cols [64,144) -> tile cols [0, 80)
        #   q -> tile m=q-1: A cols [0, 80)  -> tile cols [64, 144)
        plan = []
        for q in range(NQ):
            if q - 1 >= 0 and q - 1 < 4:
                plan.append((q, q - 1, 0, 80, 64, 144))
            if q < 4:
                plan.append((q, q, 64, 144, 0, 80))
        last_for_m = {}
        for idx, (q, m, a0, a1, t0, t1) in enumerate(plan):
            last_for_m[m] = idx
        for idx, (q, m, a0, a1, t0, t1) in enumerate(plan):
            nc.tensor.matmul(
                rowt[m][0:1, t0:t1],
                xcol[:, q:q + 1],
                A[:, q, a0:a1],
                start=False, stop=(last_for_m[m] == idx),
                skip_group_check=True)

        ysb = work.tile([1, ROWS], F32, tag="ysb")
        nc.vector.tensor_scalar(ysb[0:1, 0:128], rowt[0][0:1, 8:136],
                                0.0, None, ALU.add)
        nc.scalar.copy(ysb[0:1, 128:256], rowt[1][0:1, 8:136])
        nc.vector.tensor_scalar(ysb[0:1, 256:384], rowt[2][0:1, 8:136],
                                0.0, None, ALU.add)
        nc.scalar.copy(ysb[0:1, 384:512], rowt[3][0:1, 8:136])
        nc.scalar.dma_start(y_d[:, :], ysb[:, :])


_CACHE = {}


def build():
    if "nc" in _CACHE:
        return _CACHE["nc"]
    nc = bacc.Bacc("TRN2", target_bir_lowering=False, debug=False)
    pk1 = nc.dram_tensor("pk1", [20, 134], F32, kind="ExternalInput").ap()
    pk2 = nc.dram_tensor("pk2", [P, 9], F32, kind="ExternalInput").ap()
    y = nc.dram_tensor("y", [1, ROWS], F32, kind="ExternalOutput").ap()
    with tile.TileContext(nc) as tc:
        _emit(tc, nc, pk1, pk2, y)
    nc.compile()
    _CACHE["nc"] = nc
    return nc


def make_in_maps(x, conv_w, conv_b, W):
    xf = np.ascontiguousarray(x, dtype=np.float32).reshape(-1)
    assert xf.shape[0] == N
    cw = np.asarray(conv_w, dtype=np.float32).reshape(-1)
    cb = np.asarray(conv_b, dtype=np.float32).reshape(-1)[0]
    Wf = np.asarray(W, dtype=np.float32).reshape(-1)
    cks = np.array(
        [Wf[2], Wf[1] - 2 * Wf[2], Wf[0] - 2 * Wf[1] + Wf[2], Wf[1] - 2 * Wf[0]],
        dtype=np.float32)

    # shifted/masked copies of x (host-side slicing/padding only)
    jg = np.arange(-64 + 0, N + 576 - 512 + 0)  # covers all cores' jl ranges
    xm = np.zeros(N + 2, dtype=np.float32)
    xm[1:N + 1] = xf
    x0g = xm[1:]                                  # x[j] padded at j=N
    xm1g = np.where((np.arange(N + 1) % 1024) != 0, xm[:N + 1], 0.0)
    xp1g = np.zeros(N + 1, dtype=np.float32)
    xp1g[:N] = np.where((np.arange(N) % 1024) != 1023,
                        np.concatenate([xf[1:], [0.0]]), 0.0)

    def at(arr, j):
        j = np.asarray(j)
        v = np.where((j >= 0) & (j < N), arr[np.clip(j, 0, N - 1)], 0.0)
        return v.astype(np.float32)

    cwd = np.zeros((20, 6), dtype=np.float32)
    for q in range(NQ):
        cwd[4 * q + 0, q] = cw[0]
        cwd[4 * q + 1, q] = cw[1]
        cwd[4 * q + 2, q] = cw[2]
        cwd[4 * q + 3, q] = cb

    in_maps = []
    t = np.arange(P)
    for d in range(NCORES):
        pk1 = np.zeros((20, 134), dtype=np.float32)
        pk2 = np.zeros((P, 9), dtype=np.float32)
        for q in range(NQ):
            j = 512 * d + 128 * q + t - 64
            pk1[4 * q + 0, 0:128] = at(xm1g[:N], j)
            pk1[4 * q + 1, 0:128] = at(xf, j)
            pk1[4 * q + 2, 0:128] = at(xp1g[:N], j)
            pk1[4 * q + 3, 0:128] = 1.0
            pk2[:, q] = at(xf, j)
        pk1[:, 128:134] = cwd
        pk2[:, NQ:NQ + 4] = cks[None, :]
        in_maps.append({"pk1": pk1, "pk2": pk2})
    return in_maps


def run(x, conv_w, conv_b, W, trace=False, **kw):
    nc = build()
    in_maps = make_in_maps(x, conv_w, conv_b, W)
    res = run_bass_kernel_spmd(
        nc, in_maps, core_ids=list(range(NCORES)), trace=trace, **kw)
    y = np.concatenate([res.results[d]["y"].ravel() for d in range(NCORES)])
    return y.reshape(np.asarray(x).shape).astype(np.float32), res


def kernel(x, conv_w, conv_b, W):
    y, _ = run(x, conv_w, conv_b, W)
    return y
